# revision 39
# baseline (speedup 1.0000x reference)
"""Distributed GQA attention kernel for Trainium2 (8 NeuronCores).

Module: B=4, S=2048, H=576, 9 Q heads / 3 KV heads, HD=64, RoPE, causal
softmax, output projection.

Sharding: core c handles batch c//2 and four 256-row query blocks
({0,3,4,7} for even c, {1,2,5,6} for odd c) -- causal work is balanced at
18 key-tile units per core. Every core computes its batch's full K/V
projection locally (duplicated across the 2 cores of a batch; cheaper
than an all-gather). One SPMD graph for all 8 cores: per-slot key-tile
extents are padded to [4,8,12,16] and the causal mask is applied from
per-core mask DATA on the last 4 key-tiles of each slot.

v2: bf16 on SBUF throughout (f32 PSUM accumulate), head-pairs packed
into 128 partitions for proj+RoPE, single [65,768] PV accumulator with
one reciprocal+broadcast per (block,group), no SBUF->SBUF DMAs
(cross-partition DVE writes), Wv stored [H,195] so PV lhsT slices need
no per-group copies, output staged bf16.
"""

import sys

if "/opt/trn_rl_repo" not in sys.path:
    sys.path.insert(0, "/opt/trn_rl_repo")

import numpy as np

B, S, H = 4, 2048, 576
NH, NKV, HD = 9, 3, 64
BLK = 256           # query block rows
KT = 128            # key tile rows
EXT = [4, 8, 12, 16]  # padded key-tile extent per block slot
BLOCKS_EVEN = [0, 3, 4, 7]
BLOCKS_ODD = [1, 2, 5, 6]
HK = [128, 128, 128, 128, 64]  # contraction tiles over H=576
CH = 1024           # kv chunk width (tokens)

_CACHED = {}


def _build(reps=1):
    from concourse import bacc, bass, mybir, tile

    f32 = mybir.dt.float32
    bf16 = mybir.dt.bfloat16
    AF = mybir.ActivationFunctionType
    ALU = mybir.AluOpType

    nc = bacc.Bacc("TRN2", target_bir_lowering=False, debug=False)

    # ---- per-core inputs (bf16 unless noted) ----
    # Wall columns: [0:576]=Wq (1/8 folded), [576:768]=Wk, [768:963]=Wv65,
    # [963:1539]=Wo
    WQ0, WK0, WV0, WO0 = 0, 576, 768, 963
    xT = nc.dram_tensor("xT", [H, S], bf16, kind="ExternalInput")
    xTq = nc.dram_tensor("xTq", [H, 4 * BLK], bf16, kind="ExternalInput")
    Wall = nc.dram_tensor("Wall", [H, 1539], bf16, kind="ExternalInput")
    P2 = nc.dram_tensor("P2", [128, 128], bf16, kind="ExternalInput")  # blockdiag rot
    cosk = nc.dram_tensor("cosk", [128, S], bf16, kind="ExternalInput")  # 2-stacked
    sink = nc.dram_tensor("sink", [128, S], bf16, kind="ExternalInput")
    cosq = nc.dram_tensor("cosq", [128, 4 * BLK], bf16, kind="ExternalInput")
    sinq = nc.dram_tensor("sinq", [128, 4 * BLK], bf16, kind="ExternalInput")
    maskst = nc.dram_tensor("maskst", [4, KT, 4 * BLK], bf16, kind="ExternalInput")
    out = nc.dram_tensor("out", [4 * BLK, H], bf16, kind="ExternalOutput")

    with tile.TileContext(nc) as tc:
        with (
            tc.tile_pool(name="consts", bufs=1) as cp,
            tc.tile_pool(name="xstream", bufs=2) as xsp,
            tc.tile_pool(name="kvres", bufs=1) as kvres,
            tc.tile_pool(name="qtp", bufs=1) as qtp,
            tc.tile_pool(name="work", bufs=2) as wp,
            tc.tile_pool(name="expp", bufs=4) as expp,
            tc.tile_pool(name="mskp", bufs=4) as mskp,
            tc.tile_pool(name="ctp", bufs=2) as ctp,
            tc.tile_pool(name="outp", bufs=2) as outp,
            tc.tile_pool(name="scp", bufs=2, space="PSUM") as scp,
            tc.tile_pool(name="acp", bufs=1, space="PSUM") as acp,
        ):
            # ---- load constants (5 merged weight DMAs + 4 cos/sin + P2) ----
            Wall_sb = []
            r0 = 0
            for kt, hk in enumerate(HK):
                t = cp.tile([hk, 1539], bf16, tag=f"wall{r0}", name=f"wall{r0}")
                nc.sync.dma_start(t[:], Wall.ap()[r0 : r0 + hk, :])
                Wall_sb.append(t)
                r0 += hk
            Wq_sb = [t[:, WQ0 : WQ0 + 576] for t in Wall_sb]
            Wk_sb = [t[:, WK0 : WK0 + 192] for t in Wall_sb]
            Wv_sb = [t[:, WV0 : WV0 + 195] for t in Wall_sb]
            Wo_sb = [t[:, WO0 : WO0 + 576] for t in Wall_sb]
            P2_sb = cp.tile([128, 128], bf16, tag="P2")
            nc.scalar.dma_start(P2_sb[:], P2.ap())
            # pre-stacked pair cos/sin (rows duplicated host-side)
            cos2k = cp.tile([128, S], bf16, tag="cos2k")
            sin2k = cp.tile([128, S], bf16, tag="sin2k")
            cos2q = cp.tile([128, 4 * BLK], bf16, tag="cos2q")
            sin2q = cp.tile([128, 4 * BLK], bf16, tag="sin2q")
            for t, d in ((cos2k, cosk), (sin2k, sink), (cos2q, cosq), (sin2q, sinq)):
                nc.scalar.dma_start(t[:], d.ap())
            # trigger the exp ACT-table load during the startup DMA wait
            warm = cp.tile([1, 1], f32, tag="warm")
            nc.scalar.activation(warm[:], P2_sb[0:1, 0:1], AF.Exp)

            def one_pass():
                # K layout: pair tile [128, S] = heads (g0,g1) stacked, single [64, S]
                kTp = kvres.tile([128, S], bf16, tag="kTp", name="kTp")
                kTs = kvres.tile([HD, S], bf16, tag="kTs", name="kTs")
                # V+ones layout: [128, g*1040 + st*65 + c] (3 groups x 16 subtiles x 65)
                v_aug = kvres.tile([128, 3 * 16 * 65], bf16, tag="vaug", name="v_aug")

                def rope_raw(ps, rows, w, rtag, on_act=False):
                    raw = wp.tile([rows, w], bf16, tag=rtag, name="raw")
                    if on_act:
                        # ACT is idle pre-attention; frees DVE for the rope chain
                        nc.scalar.copy(raw[:], ps)
                    else:
                        nc.vector.tensor_copy(raw[:], ps)
                    return raw

                def rope_rot(raw, rows, w, pool, tag):
                    rot = pool.tile([rows, w], f32, tag=tag, name="rot")
                    for hf in range(0, w, 512):
                        nc.tensor.matmul(
                            rot[:, hf : hf + 512],
                            P2_sb[0:rows, 0:rows],
                            raw[:, hf : hf + 512],
                            start=True,
                            stop=True,
                        )
                    return rot

                def rope_fin(raw, rot, rows, cos_sb, sin_sb, co, w, outs):
                    t1 = wp.tile([rows, w], bf16, tag="t1", name="t1")
                    nc.vector.tensor_tensor(
                        t1[:], raw[:], cos_sb[0:rows, co : co + w], ALU.mult
                    )
                    t2 = wp.tile([rows, w], bf16, tag="t2", name="t2")
                    nc.vector.tensor_tensor(
                        t2[:], rot[:], sin_sb[0:rows, co : co + w], ALU.mult
                    )
                    for dst, r0_, r1_ in outs:
                        nc.vector.tensor_tensor(
                            dst, t1[r0_:r1_, :], t2[r0_:r1_, :], ALU.add
                        )

                # ---- K/V projection, streaming xT in 1024-token chunks ----
                def kv_chunk(ch):
                    c0 = ch * CH
                    xch = []
                    r0 = 0
                    for kt, hk in enumerate(HK):
                        t = xsp.tile([hk, CH], bf16, tag=f"xch{kt}", name=f"xch{kt}")
                        nc.sync.dma_start(t[:], xT.ap()[r0 : r0 + hk, c0 : c0 + CH])
                        xch.append(t)
                        r0 += hk
                    # K pair (g0,g1): stationary Wk[:, 0:128]
                    kp = scp.tile([128, CH], f32, tag="sc", name="kp")
                    for hf in range(2):
                        for kt in range(5):
                            nc.tensor.matmul(
                                kp[:, hf * 512 : (hf + 1) * 512],
                                Wk_sb[kt][:, 0:128],
                                xch[kt][:, hf * 512 : (hf + 1) * 512],
                                start=(kt == 0),
                                stop=(kt == 4),
                            )
                    # K single (g2): stationary Wk[:, 128:192]
                    ks = acp.tile([HD, CH], f32, tag="acc", name="ks")
                    for hf in range(2):
                        for kt in range(5):
                            nc.tensor.matmul(
                                ks[:, hf * 512 : (hf + 1) * 512],
                                Wk_sb[kt][:, 128:192],
                                xch[kt][:, hf * 512 : (hf + 1) * 512],
                                start=(kt == 0),
                                stop=(kt == 4),
                            )

                    # raw copies first (frees kp/ks PSUM slots for the V stream)
                    raw_p = rope_raw(kp[:], 128, CH, "rawp")
                    raw_s = rope_raw(ks[:], HD, CH, "raws")

                    # V: 8 key-subtiles of 128 tokens, out [128, 195]
                    for st8 in range(8):
                        st = ch * 8 + st8
                        vps = scp.tile([128, 3 * 65], f32, tag="sc", name="vps")
                        for kt in range(5):
                            nc.tensor.matmul(
                                vps[:],
                                xch[kt][:, st8 * 128 : (st8 + 1) * 128],
                                Wv_sb[kt][:],
                                start=(kt == 0),
                                stop=(kt == 4),
                            )
                        dst = v_aug[:].rearrange("p (g s c) -> p g s c", g=3, c=65)[
                            :, :, st, :
                        ]
                        nc.vector.tensor_copy(
                            dst, vps[:].rearrange("p (g c) -> p g c", c=65)
                        )

                    rot_p = rope_rot(raw_p, 128, CH, scp, "sc")
                    rot_s = rope_rot(raw_s, HD, CH, acp, "acc")
                    rope_fin(raw_p, rot_p, 128, cos2k, sin2k, c0, CH,
                             [(kTp[:, c0 : c0 + CH], 0, 128)])
                    rope_fin(raw_s, rot_s, HD, cos2k, sin2k, c0, CH,
                             [(kTs[:, c0 : c0 + CH], 0, HD)])

                # ---- Q proj + RoPE; layout qT2 [64, j*2304 + h*256] (block-major)
                # Q lives at partitions 0:64 for groups 0,2 and 64:128 for group 1
                # (matching the packed-K lhsT base so matmul bases line up).
                xq = []
                qT2 = qtp.tile([128, 4 * NH * BLK], bf16, tag="qT2", name="qT2")

                def load_xq():
                    r0 = 0
                    for kt, hk in enumerate(HK):
                        t = qtp.tile([hk, 4 * BLK], bf16, tag=f"xq{kt}", name=f"xq{kt}")
                        nc.scalar.dma_start(t[:], xTq.ap()[r0 : r0 + hk, :])
                        xq.append(t)
                        r0 += hk

                def qdst(h, hf):
                    # strided dst: blocks {2hf, 2hf+1}, head h
                    pb = 64 if h // 3 == 1 else 0
                    v = qT2[pb : pb + HD, :].rearrange(
                        "p (j h c) -> p j h c", j=4, h=NH
                    )
                    return v[:, 2 * hf : 2 * hf + 2, h, :]

                def q_half(hf):
                    cq0 = hf * 512
                    # 3 true pairs + packed singles (2,5) + single 8
                    packs = []  # (raw, rows, [(head, r0, r1)], psum_pool, tag)
                    for pi, (h0, h1) in enumerate(((0, 1), (3, 4), (6, 7))):
                        qp = scp.tile([128, 512], f32, tag="sc", name="qp")
                        for kt in range(5):
                            nc.tensor.matmul(
                                qp[:],
                                Wq_sb[kt][:, h0 * HD : h0 * HD + 128],
                                xq[kt][:, cq0 : cq0 + 512],
                                start=(kt == 0),
                                stop=(kt == 4),
                            )
                        raw = rope_raw(qp[:], 128, 512, f"qr{pi}")
                        packs.append((raw, 128, [(h0, 0, 64), (h1, 64, 128)], scp, "sc"))
                    qs = acp.tile([128, 512], f32, tag="acc", name="qs")
                    for half, h in ((0, 2), (64, 5)):
                        for kt in range(5):
                            nc.tensor.matmul(
                                qs[half : half + 64, :],
                                Wq_sb[kt][:, h * HD : (h + 1) * HD],
                                xq[kt][:, cq0 : cq0 + 512],
                                start=(kt == 0),
                                stop=(kt == 4),
                            )
                    packs.append((rope_raw(qs[:], 128, 512, "qr3"), 128,
                                  [(2, 0, 64), (5, 64, 128)], acp, "acc"))
                    q8 = acp.tile([HD, 512], f32, tag="acc", name="q8")
                    for kt in range(5):
                        nc.tensor.matmul(
                            q8[:],
                            Wq_sb[kt][:, 8 * HD : 9 * HD],
                            xq[kt][:, cq0 : cq0 + 512],
                            start=(kt == 0),
                            stop=(kt == 4),
                        )
                    packs.append((rope_raw(q8[:], HD, 512, "qr4"), HD,
                                  [(8, 0, 64)], acp, "acc"))
                    rots = [
                        rope_rot(raw, rows, 512, pool, tag)
                        for raw, rows, _, pool, tag in packs
                    ]
                    for (raw, rows, heads, _, _), rot in zip(packs, rots):
                        outs = [(qdst(h, hf), r0_, r1_) for h, r0_, r1_ in heads]
                        rope_fin(raw, rot, rows, cos2q, sin2q, cq0, 512, outs)

                # interleave chunks/halves so attention on block 0 can start
                # while chunk-1 K/V and Q-half-1 still compute
                def vones(ch):
                    # ones column (col 64 of each 65-block) for the row-sum
                    dst = v_aug[:].rearrange("p (g s c) -> p g s c", g=3, c=65)[
                        :, :, ch * 8 : ch * 8 + 8, 64:65
                    ]
                    nc.gpsimd.memset(dst, 1.0)

                # ---- attention ----
                def emit_wo(j, cts):
                    for half in range(2):
                        h0r = half * 128
                        wo = scp.tile([128, H], f32, tag="sc", name="wo")
                        for t in range(5):
                            lhsT = cts[t][:, h0r : h0r + 128]
                            nc.tensor.matmul(
                                wo[:, 0:512], lhsT, Wo_sb[t][:, 0:512],
                                start=(t == 0), stop=(t == 4),
                            )
                            nc.tensor.matmul(
                                wo[:, 512:576], lhsT, Wo_sb[t][:, 512:576],
                                start=(t == 0), stop=(t == 4),
                            )
                        osb = outp.tile([128, H], bf16, tag="osb", name="osb")
                        nc.vector.tensor_copy(osb[:], wo[:])
                        nc.sync.dma_start(
                            out.ap()[j * BLK + h0r : j * BLK + h0r + 128, :], osb[:]
                        )

                state = {"wo_pending": None}

                def attention_block(j):
                    ext = EXT[j]
                    mt = mts[j]
                    cts = [
                        ctp.tile([128, BLK], bf16, tag=f"ct{t}", name=f"ct{t}")
                        for t in range(4)
                    ]
                    cts.append(ctp.tile([HD, BLK], bf16, tag="ct4", name="ct4"))
                    for g in range(NKV):
                        h0 = 3 * g
                        acc = acp.tile([65, 3 * BLK], f32, tag="acc", name="acc")
                        pb = 64 if g == 1 else 0
                        qvj = qT2[pb : pb + HD, j * NH * BLK : (j + 1) * NH * BLK]
                        qpair = qvj[:, h0 * BLK : (h0 + 2) * BLK]
                        qsng = qvj[:, (h0 + 2) * BLK : (h0 + 3) * BLK]
                        for kp2 in range(ext // 2):
                            kcL, kcR = 2 * kp2, 2 * kp2 + 1

                            def ktile(kc):
                                return (
                                    kTp[g * HD : (g + 1) * HD,
                                        kc * KT : (kc + 1) * KT]
                                    if g < 2
                                    else kTs[:, kc * KT : (kc + 1) * KT]
                                )

                            # bank layout of the 6KB pair tile:
                            # [L-pair 0:512 | L-h3 512:768, R-h3 768:1024 |
                            #  R-pair 1024:1536] -- each matmul within a bank
                            sps = scp.tile([KT, 6 * BLK], f32, tag="sc",
                                           name="sps")
                            nc.tensor.matmul(sps[:, 0:512], ktile(kcL), qpair,
                                             start=True, stop=True)
                            nc.tensor.matmul(sps[:, 512:768], ktile(kcL), qsng,
                                             start=True, stop=True)
                            nc.tensor.matmul(sps[:, 768:1024], ktile(kcR),
                                             qsng, start=True, stop=True)
                            nc.tensor.matmul(sps[:, 1024:1536], ktile(kcR),
                                             qpair, start=True, stop=True)
                            esb = expp.tile([KT, 6 * BLK], bf16, tag="exp",
                                            name="esb")
                            nc.scalar.activation(esb[:], sps[:], AF.Exp)
                            for kc, cbase in ((kcL, 0), (kcR, 768)):
                                if kc >= ext - 4:
                                    off = kc - (ext - 4)
                                    esv = esb[:, cbase : cbase + 768].rearrange(
                                        "p (i c) -> p i c", i=3
                                    )
                                    msl = (
                                        mt[:, off * BLK : (off + 1) * BLK]
                                        .unsqueeze(1)
                                        .broadcast_to([KT, 3, BLK])
                                    )
                                    nc.gpsimd.tensor_tensor(esv, esv, msl,
                                                            ALU.mult)
                            vL = v_aug[:, (g * 16 + kcL) * 65 :
                                       (g * 16 + kcL) * 65 + 65]
                            vR = v_aug[:, (g * 16 + kcR) * 65 :
                                       (g * 16 + kcR) * 65 + 65]
                            last = kp2 == ext // 2 - 1
                            nc.tensor.matmul(acc[:, 0:512], vL, esb[:, 0:512],
                                             start=(kp2 == 0), stop=False)
                            nc.tensor.matmul(acc[:, 512:768], vL,
                                             esb[:, 512:768],
                                             start=(kp2 == 0), stop=False)
                            nc.tensor.matmul(acc[:, 0:512], vR,
                                             esb[:, 1024:1536], start=False,
                                             stop=last)
                            nc.tensor.matmul(acc[:, 512:768], vR,
                                             esb[:, 768:1024], start=False,
                                             stop=last)
                        # normalize: 1/rowsum broadcast, scale into cts
                        rec = wp.tile([1, 3 * BLK], f32, tag="rec", name="rec")
                        nc.vector.reciprocal(rec[0:1, :], acc[64:65, :])
                        bc = wp.tile([HD, 3 * BLK], f32, tag="bc", name="bc")
                        nc.gpsimd.partition_broadcast(bc[:], rec[0:1, :])
                        for i in range(3):
                            h = h0 + i
                            t, lo = divmod(h, 2)
                            nc.vector.tensor_tensor(
                                cts[t][lo * HD : lo * HD + HD, :],
                                acc[0:HD, i * BLK : (i + 1) * BLK],
                                bc[:, i * BLK : (i + 1) * BLK],
                                ALU.mult,
                            )
                        # previous block's out-projection, emitted here so the
                        # in-order PE queue has this block's scores/PV queued
                        # ahead of Wo's cts dependency
                        if g == 0 and state["wo_pending"] is not None:
                            state["wo_pending"]()
                            state["wo_pending"] = None

                    state["wo_pending"] = (
                        lambda jj, cc: lambda: emit_wo(jj, cc)
                    )(j, cts)

                kv_chunk(0)
                vones(0)
                load_xq()
                q_half(0)
                mts = []
                for j in range(4):
                    mt = mskp.tile([KT, 4 * BLK], bf16, tag="msk", name="msk")
                    nc.scalar.dma_start(mt[:], maskst.ap()[j, :, :])
                    mts.append(mt)
                kv_chunk(1)
                vones(1)
                q_half(1)
                for j in range(4):
                    attention_block(j)
                if state["wo_pending"] is not None:
                    state["wo_pending"]()

            for _rep in range(reps):
                one_pass()

    nc.compile()
    return nc


def _get_nc(reps=1):
    key = f"nc{reps}"
    if key not in _CACHED:
        _CACHED[key] = _build(reps=reps)
    return _CACHED[key]


def _make_in_maps(x, cos, sin, mask, Wq, Wk, Wv, Wo):
    import ml_dtypes

    f4 = np.float32
    bf = ml_dtypes.bfloat16
    Wv65 = np.zeros((H, 3 * 65), f4)
    for g in range(3):
        Wv65[:, g * 65 : g * 65 + 64] = Wv[:, g * 64 : (g + 1) * 64]
    P2 = np.zeros((128, 128), f4)
    half = HD // 2
    for base in (0, 64):
        for m in range(half):
            P2[base + m + half, base + m] = -1.0
        for m in range(half, HD):
            P2[base + m - half, base + m] = 1.0
    cosT = np.ascontiguousarray(cos.T.astype(f4))  # [64, S]
    sinT = np.ascontiguousarray(sin.T.astype(f4))
    scale = np.float32(1.0 / np.sqrt(HD))
    maskT_full = np.ascontiguousarray(mask[0, 0].T.astype(f4))  # [k, q]
    Wall = np.concatenate(
        [Wq.astype(f4) * scale, Wk.astype(f4), Wv65, Wo.astype(f4)], axis=1
    )  # [576, 1539]
    cosk2 = np.concatenate([cosT, cosT], 0)  # [128, S]
    sink2 = np.concatenate([sinT, sinT], 0)

    in_maps = []
    for c in range(8):
        b = c // 2
        blocks = BLOCKS_EVEN if c % 2 == 0 else BLOCKS_ODD
        xb = x[b]  # [S, H]
        xTc = np.ascontiguousarray(xb.T.astype(f4))  # [H, S]
        qcols = np.concatenate(
            [xTc[:, blk * BLK : (blk + 1) * BLK] for blk in blocks], axis=1
        )
        cosqc = np.concatenate(
            [cosT[:, blk * BLK : (blk + 1) * BLK] for blk in blocks], axis=1
        )
        sinqc = np.concatenate(
            [sinT[:, blk * BLK : (blk + 1) * BLK] for blk in blocks], axis=1
        )
        maskstk = np.empty((4, KT, 4 * BLK), f4)
        for j, blk in enumerate(blocks):
            ext = EXT[j]
            for off in range(4):
                kc = ext - 4 + off
                sl = maskT_full[kc * KT : (kc + 1) * KT, blk * BLK : (blk + 1) * BLK]
                maskstk[j, :, off * BLK : (off + 1) * BLK] = (sl > -1.0).astype(f4)
        in_maps.append(
            {
                "xT": xTc.astype(bf),
                "xTq": np.ascontiguousarray(qcols).astype(bf),
                "Wall": Wall.astype(bf),
                "P2": P2.astype(bf),
                "cosk": cosk2.astype(bf),
                "sink": sink2.astype(bf),
                "cosq": np.concatenate([cosqc, cosqc], 0).astype(bf),
                "sinq": np.concatenate([sinqc, sinqc], 0).astype(bf),
                "maskst": maskstk.astype(bf),
            }
        )
    return in_maps


def kernel(x, cos, sin, mask, Wq, Wk, Wv, Wo, _trace=False, _trace_kwargs=None):
    from concourse import bass_utils

    x = np.asarray(x)
    in_maps = _make_in_maps(
        np.asarray(x), np.asarray(cos), np.asarray(sin), np.asarray(mask),
        np.asarray(Wq), np.asarray(Wk), np.asarray(Wv), np.asarray(Wo),
    )
    nc = _get_nc()
    kw = {}
    if _trace:
        kw["trace"] = True
        if _trace_kwargs:
            kw.update(_trace_kwargs)
    res = bass_utils.run_bass_kernel_spmd(nc, in_maps, core_ids=list(range(8)), **kw)
    out = np.empty((B, S, H), np.float32)
    for c in range(8):
        b = c // 2
        blocks = BLOCKS_EVEN if c % 2 == 0 else BLOCKS_ODD
        o = np.asarray(res.results[c]["out"]).astype(np.float32)  # [1024, 576]
        for j, blk in enumerate(blocks):
            out[b, blk * BLK : (blk + 1) * BLK, :] = o[j * BLK : (j + 1) * BLK, :]
    if _trace:
        _CACHED["last_result"] = res
    return out


# revision 41
# speedup vs baseline: 2.8711x; 2.8711x over previous
"""Distributed GQA attention kernel for Trainium2 (8 NeuronCores).

Module: B=4, S=2048, H=576, 9 Q heads / 3 KV heads, HD=64, RoPE, causal
softmax, output projection.

Sharding: core c handles batch c//2 and four 256-row query blocks
({0,3,4,7} for even c, {1,2,5,6} for odd c) -- causal work is balanced at
18 key-tile units per core. Every core computes its batch's full K/V
projection locally (duplicated across the 2 cores of a batch; cheaper
than an all-gather). One SPMD graph for all 8 cores: per-slot key-tile
extents are padded to [4,8,12,16] and the causal mask is applied from
per-core mask DATA on the last 4 key-tiles of each slot.

v2: bf16 on SBUF throughout (f32 PSUM accumulate), head-pairs packed
into 128 partitions for proj+RoPE, single [65,768] PV accumulator with
one reciprocal+broadcast per (block,group), no SBUF->SBUF DMAs
(cross-partition DVE writes), Wv stored [H,195] so PV lhsT slices need
no per-group copies, output staged bf16.
"""

import sys

if "/opt/trn_rl_repo" not in sys.path:
    sys.path.insert(0, "/opt/trn_rl_repo")

import numpy as np

B, S, H = 4, 2048, 576
NH, NKV, HD = 9, 3, 64
BLK = 256           # query block rows
KT = 128            # key tile rows
EXT = [4, 8, 12, 16]  # padded key-tile extent per block slot
BLOCKS_EVEN = [0, 3, 4, 7]
BLOCKS_ODD = [1, 2, 5, 6]
HK = [128, 128, 128, 128, 64]  # contraction tiles over H=576
CH = 1024           # kv chunk width (tokens)

_CACHED = {}


def _build(reps=1):
    from concourse import bacc, bass, mybir, tile

    f32 = mybir.dt.float32
    bf16 = mybir.dt.bfloat16
    AF = mybir.ActivationFunctionType
    ALU = mybir.AluOpType

    nc = bacc.Bacc("TRN2", target_bir_lowering=False, debug=False)

    # ---- per-core inputs (bf16 unless noted) ----
    # Wall columns: [0:576]=Wq (1/8 folded), [576:768]=Wk, [768:963]=Wv65,
    # [963:1539]=Wo
    WQ0, WK0, WV0, WO0 = 0, 576, 768, 963
    xT = nc.dram_tensor("xT", [H, S], bf16, kind="ExternalInput")
    xTq = nc.dram_tensor("xTq", [H, 4 * BLK], bf16, kind="ExternalInput")
    Wall = nc.dram_tensor("Wall", [H, 1539], bf16, kind="ExternalInput")
    P2 = nc.dram_tensor("P2", [128, 128], bf16, kind="ExternalInput")  # blockdiag rot
    cosk = nc.dram_tensor("cosk", [128, S], bf16, kind="ExternalInput")  # 2-stacked
    sink = nc.dram_tensor("sink", [128, S], bf16, kind="ExternalInput")
    cosq = nc.dram_tensor("cosq", [128, 4 * BLK], bf16, kind="ExternalInput")
    sinq = nc.dram_tensor("sinq", [128, 4 * BLK], bf16, kind="ExternalInput")
    maskst = nc.dram_tensor("maskst", [4, KT, 4 * BLK], bf16, kind="ExternalInput")
    out = nc.dram_tensor("out", [4 * BLK, H], bf16, kind="ExternalOutput")

    with tile.TileContext(nc) as tc:
        with (
            tc.tile_pool(name="consts", bufs=1) as cp,
            tc.tile_pool(name="xstream", bufs=2) as xsp,
            tc.tile_pool(name="kvres", bufs=1) as kvres,
            tc.tile_pool(name="qtp", bufs=1) as qtp,
            tc.tile_pool(name="work", bufs=2) as wp,
            tc.tile_pool(name="expp", bufs=4) as expp,
            tc.tile_pool(name="mskp", bufs=4) as mskp,
            tc.tile_pool(name="ctp", bufs=2) as ctp,
            tc.tile_pool(name="outp", bufs=2) as outp,
            tc.tile_pool(name="scp", bufs=2, space="PSUM") as scp,
            tc.tile_pool(name="acp", bufs=2, space="PSUM") as acp,
        ):
            # ---- load constants (5 merged weight DMAs + 4 cos/sin + P2) ----
            # Wall DMAs are deferred and interleaved with the first x-chunk's
            # DMAs on the sync ring so K-proj matmul kt can start after 2(kt+1)
            # DMAs instead of all 10.
            Wall_sb = []
            wall_dmas = []
            r0 = 0
            for kt, hk in enumerate(HK):
                t = cp.tile([hk, 1539], bf16, tag=f"wall{r0}", name=f"wall{r0}")
                wall_dmas.append((t, r0, hk))
                Wall_sb.append(t)
                r0 += hk
            Wq_sb = [t[:, WQ0 : WQ0 + 576] for t in Wall_sb]
            Wk_sb = [t[:, WK0 : WK0 + 192] for t in Wall_sb]
            Wv_sb = [t[:, WV0 : WV0 + 195] for t in Wall_sb]
            Wo_sb = [t[:, WO0 : WO0 + 576] for t in Wall_sb]
            P2_sb = cp.tile([128, 128], bf16, tag="P2")
            nc.scalar.dma_start(P2_sb[:], P2.ap())
            # pre-stacked pair cos/sin (rows duplicated host-side)
            cos2k = cp.tile([128, S], bf16, tag="cos2k")
            sin2k = cp.tile([128, S], bf16, tag="sin2k")
            cos2q = cp.tile([128, 4 * BLK], bf16, tag="cos2q")
            sin2q = cp.tile([128, 4 * BLK], bf16, tag="sin2q")
            for t, d in ((cos2k, cosk), (sin2k, sink), (cos2q, cosq), (sin2q, sinq)):
                nc.scalar.dma_start(t[:], d.ap())
            # trigger the exp ACT-table load during the startup DMA wait
            warm = cp.tile([1, 1], f32, tag="warm")
            nc.scalar.activation(warm[:], P2_sb[0:1, 0:1], AF.Exp)

            def one_pass():
                # K layout: pair tile [128, S] = heads (g0,g1) stacked, single [64, S]
                kTp = kvres.tile([128, S], bf16, tag="kTp", name="kTp")
                kTs = kvres.tile([HD, S], bf16, tag="kTs", name="kTs")
                # V+ones layout: [128, g*1040 + st*65 + c] (3 groups x 16 subtiles x 65)
                v_aug = kvres.tile([128, 3 * 16 * 65], bf16, tag="vaug", name="v_aug")

                def rope_raw(ps, rows, w, rtag, on_act=False):
                    raw = wp.tile([rows, w], bf16, tag=rtag, name="raw")
                    if on_act:
                        # ACT is idle pre-attention; frees DVE for the rope chain
                        nc.scalar.copy(raw[:], ps)
                    else:
                        nc.vector.tensor_copy(raw[:], ps)
                    return raw

                def rope_rot(raw, rows, w, pool, tag):
                    rot = pool.tile([rows, w], f32, tag=tag, name="rot")
                    for hf in range(0, w, 512):
                        nc.tensor.matmul(
                            rot[:, hf : hf + 512],
                            P2_sb[0:rows, 0:rows],
                            raw[:, hf : hf + 512],
                            start=True,
                            stop=True,
                        )
                    return rot

                def rope_fin(raw, rot, rows, cos_sb, sin_sb, co, w, outs):
                    t1 = wp.tile([rows, w], bf16, tag="t1", name="t1")
                    nc.vector.tensor_tensor(
                        t1[:], raw[:], cos_sb[0:rows, co : co + w], ALU.mult
                    )
                    t2 = wp.tile([rows, w], bf16, tag="t2", name="t2")
                    nc.vector.tensor_tensor(
                        t2[:], rot[:], sin_sb[0:rows, co : co + w], ALU.mult
                    )
                    for dst, r0_, r1_ in outs:
                        nc.vector.tensor_tensor(
                            dst, t1[r0_:r1_, :], t2[r0_:r1_, :], ALU.add
                        )

                # ---- K/V projection, streaming xT in 1024-token chunks ----
                def kv_chunk(ch):
                    c0 = ch * CH
                    xch = []
                    r0 = 0
                    for kt, hk in enumerate(HK):
                        if wall_dmas:
                            wt, wr0, whk = wall_dmas.pop(0)
                            nc.sync.dma_start(
                                wt[:], Wall.ap()[wr0 : wr0 + whk, :]
                            )
                        t = xsp.tile([hk, CH], bf16, tag=f"xch{kt}", name=f"xch{kt}")
                        nc.sync.dma_start(t[:], xT.ap()[r0 : r0 + hk, c0 : c0 + CH])
                        xch.append(t)
                        r0 += hk
                    # K pair (g0,g1): stationary Wk[:, 0:128]
                    kp = scp.tile([128, CH], f32, tag="sc", name="kp")
                    for hf in range(2):
                        for kt in range(5):
                            nc.tensor.matmul(
                                kp[:, hf * 512 : (hf + 1) * 512],
                                Wk_sb[kt][:, 0:128],
                                xch[kt][:, hf * 512 : (hf + 1) * 512],
                                start=(kt == 0),
                                stop=(kt == 4),
                            )
                    # K single (g2): stationary Wk[:, 128:192]
                    ks = acp.tile([HD, CH], f32, tag="acc", name="ks")
                    for hf in range(2):
                        for kt in range(5):
                            nc.tensor.matmul(
                                ks[:, hf * 512 : (hf + 1) * 512],
                                Wk_sb[kt][:, 128:192],
                                xch[kt][:, hf * 512 : (hf + 1) * 512],
                                start=(kt == 0),
                                stop=(kt == 4),
                            )

                    # raw copies first (frees kp/ks PSUM slots for the V stream)
                    raw_p = rope_raw(kp[:], 128, CH, "rawp")
                    raw_s = rope_raw(ks[:], HD, CH, "raws")

                    # V: 8 key-subtiles of 128 tokens, out [128, 195]
                    for st8 in range(8):
                        st = ch * 8 + st8
                        vps = scp.tile([128, 3 * 65], f32, tag="sc", name="vps")
                        for kt in range(5):
                            nc.tensor.matmul(
                                vps[:],
                                xch[kt][:, st8 * 128 : (st8 + 1) * 128],
                                Wv_sb[kt][:],
                                start=(kt == 0),
                                stop=(kt == 4),
                            )
                        dst = v_aug[:].rearrange("p (g s c) -> p g s c", g=3, c=65)[
                            :, :, st, :
                        ]
                        nc.vector.tensor_copy(
                            dst, vps[:].rearrange("p (g c) -> p g c", c=65)
                        )

                    rot_p = rope_rot(raw_p, 128, CH, scp, "sc")
                    rot_s = rope_rot(raw_s, HD, CH, acp, "acc")
                    rope_fin(raw_p, rot_p, 128, cos2k, sin2k, c0, CH,
                             [(kTp[:, c0 : c0 + CH], 0, 128)])
                    rope_fin(raw_s, rot_s, HD, cos2k, sin2k, c0, CH,
                             [(kTs[:, c0 : c0 + CH], 0, HD)])

                # ---- Q proj + RoPE; layout qT2 [64, j*2304 + h*256] (block-major)
                # Q lives at partitions 0:64 for groups 0,2 and 64:128 for group 1
                # (matching the packed-K lhsT base so matmul bases line up).
                xq = []
                qT2 = qtp.tile([128, 4 * NH * BLK], bf16, tag="qT2", name="qT2")

                def load_xq():
                    r0 = 0
                    for kt, hk in enumerate(HK):
                        t = qtp.tile([hk, 4 * BLK], bf16, tag=f"xq{kt}", name=f"xq{kt}")
                        nc.scalar.dma_start(t[:], xTq.ap()[r0 : r0 + hk, :])
                        xq.append(t)
                        r0 += hk

                def qdst(h, hf):
                    # strided dst: blocks {2hf, 2hf+1}, head h
                    pb = 64 if h // 3 == 1 else 0
                    v = qT2[pb : pb + HD, :].rearrange(
                        "p (j h c) -> p j h c", j=4, h=NH
                    )
                    return v[:, 2 * hf : 2 * hf + 2, h, :]

                def q_half(hf):
                    cq0 = hf * 512
                    # 3 true pairs + packed singles (2,5) + single 8
                    packs = []  # (raw, rows, [(head, r0, r1)], psum_pool, tag)
                    for pi, (h0, h1) in enumerate(((0, 1), (3, 4), (6, 7))):
                        qp = scp.tile([128, 512], f32, tag="sc", name="qp")
                        for kt in range(5):
                            nc.tensor.matmul(
                                qp[:],
                                Wq_sb[kt][:, h0 * HD : h0 * HD + 128],
                                xq[kt][:, cq0 : cq0 + 512],
                                start=(kt == 0),
                                stop=(kt == 4),
                            )
                        raw = rope_raw(qp[:], 128, 512, f"qr{pi}")
                        packs.append((raw, 128, [(h0, 0, 64), (h1, 64, 128)], scp, "sc"))
                    qs = acp.tile([128, 512], f32, tag="acc", name="qs")
                    for half, h in ((0, 2), (64, 5)):
                        for kt in range(5):
                            nc.tensor.matmul(
                                qs[half : half + 64, :],
                                Wq_sb[kt][:, h * HD : (h + 1) * HD],
                                xq[kt][:, cq0 : cq0 + 512],
                                start=(kt == 0),
                                stop=(kt == 4),
                            )
                    packs.append((rope_raw(qs[:], 128, 512, "qr3"), 128,
                                  [(2, 0, 64), (5, 64, 128)], acp, "acc"))
                    q8 = acp.tile([HD, 512], f32, tag="acc", name="q8")
                    for kt in range(5):
                        nc.tensor.matmul(
                            q8[:],
                            Wq_sb[kt][:, 8 * HD : 9 * HD],
                            xq[kt][:, cq0 : cq0 + 512],
                            start=(kt == 0),
                            stop=(kt == 4),
                        )
                    packs.append((rope_raw(q8[:], HD, 512, "qr4"), HD,
                                  [(8, 0, 64)], acp, "acc"))
                    rots = [
                        rope_rot(raw, rows, 512, pool, tag)
                        for raw, rows, _, pool, tag in packs
                    ]
                    for (raw, rows, heads, _, _), rot in zip(packs, rots):
                        outs = [(qdst(h, hf), r0_, r1_) for h, r0_, r1_ in heads]
                        rope_fin(raw, rot, rows, cos2q, sin2q, cq0, 512, outs)

                # interleave chunks/halves so attention on block 0 can start
                # while chunk-1 K/V and Q-half-1 still compute
                def vones(ch):
                    # ones column (col 64 of each 65-block) for the row-sum
                    dst = v_aug[:].rearrange("p (g s c) -> p g s c", g=3, c=65)[
                        :, :, ch * 8 : ch * 8 + 8, 64:65
                    ]
                    nc.gpsimd.memset(dst, 1.0)

                # ---- attention ----
                def emit_wo(j, cts):
                    for half in range(2):
                        h0r = half * 128
                        wo = scp.tile([128, H], f32, tag="sc", name="wo")
                        for t in range(5):
                            lhsT = cts[t][:, h0r : h0r + 128]
                            nc.tensor.matmul(
                                wo[:, 0:512], lhsT, Wo_sb[t][:, 0:512],
                                start=(t == 0), stop=(t == 4),
                            )
                            nc.tensor.matmul(
                                wo[:, 512:576], lhsT, Wo_sb[t][:, 512:576],
                                start=(t == 0), stop=(t == 4),
                            )
                        osb = outp.tile([128, H], bf16, tag="osb", name="osb")
                        nc.vector.tensor_copy(osb[:], wo[:])
                        nc.sync.dma_start(
                            out.ap()[j * BLK + h0r : j * BLK + h0r + 128, :], osb[:]
                        )

                state = {"wo_pending": None}

                def attention_block(j):
                    ext = EXT[j]
                    mt = mts[j]
                    cts = [
                        ctp.tile([128, BLK], bf16, tag=f"ct{t}", name=f"ct{t}")
                        for t in range(4)
                    ]
                    cts.append(ctp.tile([HD, BLK], bf16, tag="ct4", name="ct4"))
                    for g in range(NKV):
                        h0 = 3 * g
                        acc = acp.tile([65, 3 * BLK], f32, tag="acc", name="acc")
                        pb = 64 if g == 1 else 0
                        qvj = qT2[pb : pb + HD, j * NH * BLK : (j + 1) * NH * BLK]
                        for kc in range(ext):
                            ktile = (
                                kTp[g * HD : (g + 1) * HD, kc * KT : (kc + 1) * KT]
                                if g < 2
                                else kTs[:, kc * KT : (kc + 1) * KT]
                            )
                            sps = scp.tile([KT, 3 * BLK], f32, tag="sc", name="sps")
                            nc.tensor.matmul(
                                sps[:, 0:512],
                                ktile,
                                qvj[:, h0 * BLK : (h0 + 2) * BLK],
                                start=True,
                                stop=True,
                            )
                            nc.tensor.matmul(
                                sps[:, 512:768],
                                ktile,
                                qvj[:, (h0 + 2) * BLK : (h0 + 3) * BLK],
                                start=True,
                                stop=True,
                            )
                            esb = expp.tile([KT, 3 * BLK], bf16, tag="exp", name="esb")
                            nc.scalar.activation(esb[:], sps[:], AF.Exp)
                            if kc >= ext - 4:
                                off = kc - (ext - 4)
                                esv = esb[:].rearrange("p (i c) -> p i c", i=3)
                                msl = (
                                    mt[:, off * BLK : (off + 1) * BLK]
                                    .unsqueeze(1)
                                    .broadcast_to([KT, 3, BLK])
                                )
                                nc.gpsimd.tensor_tensor(esv, esv, msl, ALU.mult)
                            vt = v_aug[:, (g * 16 + kc) * 65 : (g * 16 + kc) * 65 + 65]
                            nc.tensor.matmul(
                                acc[:, 0:512], vt, esb[:, 0:512],
                                start=(kc == 0), stop=(kc == ext - 1),
                            )
                            nc.tensor.matmul(
                                acc[:, 512:768], vt, esb[:, 512:768],
                                start=(kc == 0), stop=(kc == ext - 1),
                            )
                        # normalize: 1/rowsum broadcast, scale into cts
                        rec = wp.tile([1, 3 * BLK], f32, tag="rec", name="rec")
                        nc.vector.reciprocal(rec[0:1, :], acc[64:65, :])
                        bc = wp.tile([HD, 3 * BLK], f32, tag="bc", name="bc")
                        nc.gpsimd.partition_broadcast(bc[:], rec[0:1, :])
                        for i in range(3):
                            h = h0 + i
                            t, lo = divmod(h, 2)
                            nc.vector.tensor_tensor(
                                cts[t][lo * HD : lo * HD + HD, :],
                                acc[0:HD, i * BLK : (i + 1) * BLK],
                                bc[:, i * BLK : (i + 1) * BLK],
                                ALU.mult,
                            )
                        # previous block's out-projection, emitted here so the
                        # in-order PE queue has this block's scores/PV queued
                        # ahead of Wo's cts dependency
                        if g == 0 and state["wo_pending"] is not None:
                            state["wo_pending"]()
                            state["wo_pending"] = None

                    state["wo_pending"] = (
                        lambda jj, cc: lambda: emit_wo(jj, cc)
                    )(j, cts)

                kv_chunk(0)
                vones(0)
                load_xq()
                q_half(0)
                mts = []
                for j in range(4):
                    mt = mskp.tile([KT, 4 * BLK], bf16, tag="msk", name="msk")
                    nc.scalar.dma_start(mt[:], maskst.ap()[j, :, :])
                    mts.append(mt)
                kv_chunk(1)
                vones(1)
                q_half(1)
                for j in range(4):
                    attention_block(j)
                if state["wo_pending"] is not None:
                    state["wo_pending"]()

            for _rep in range(reps):
                one_pass()

    nc.compile()
    return nc


def _get_nc(reps=1):
    key = f"nc{reps}"
    if key not in _CACHED:
        _CACHED[key] = _build(reps=reps)
    return _CACHED[key]


def _make_in_maps(x, cos, sin, mask, Wq, Wk, Wv, Wo):
    import ml_dtypes

    f4 = np.float32
    bf = ml_dtypes.bfloat16
    Wv65 = np.zeros((H, 3 * 65), f4)
    for g in range(3):
        Wv65[:, g * 65 : g * 65 + 64] = Wv[:, g * 64 : (g + 1) * 64]
    P2 = np.zeros((128, 128), f4)
    half = HD // 2
    for base in (0, 64):
        for m in range(half):
            P2[base + m + half, base + m] = -1.0
        for m in range(half, HD):
            P2[base + m - half, base + m] = 1.0
    cosT = np.ascontiguousarray(cos.T.astype(f4))  # [64, S]
    sinT = np.ascontiguousarray(sin.T.astype(f4))
    scale = np.float32(1.0 / np.sqrt(HD))
    maskT_full = np.ascontiguousarray(mask[0, 0].T.astype(f4))  # [k, q]
    Wall = np.concatenate(
        [Wq.astype(f4) * scale, Wk.astype(f4), Wv65, Wo.astype(f4)], axis=1
    )  # [576, 1539]
    cosk2 = np.concatenate([cosT, cosT], 0)  # [128, S]
    sink2 = np.concatenate([sinT, sinT], 0)

    in_maps = []
    for c in range(8):
        b = c // 2
        blocks = BLOCKS_EVEN if c % 2 == 0 else BLOCKS_ODD
        xb = x[b]  # [S, H]
        xTc = np.ascontiguousarray(xb.T.astype(f4))  # [H, S]
        qcols = np.concatenate(
            [xTc[:, blk * BLK : (blk + 1) * BLK] for blk in blocks], axis=1
        )
        cosqc = np.concatenate(
            [cosT[:, blk * BLK : (blk + 1) * BLK] for blk in blocks], axis=1
        )
        sinqc = np.concatenate(
            [sinT[:, blk * BLK : (blk + 1) * BLK] for blk in blocks], axis=1
        )
        maskstk = np.empty((4, KT, 4 * BLK), f4)
        for j, blk in enumerate(blocks):
            ext = EXT[j]
            for off in range(4):
                kc = ext - 4 + off
                sl = maskT_full[kc * KT : (kc + 1) * KT, blk * BLK : (blk + 1) * BLK]
                maskstk[j, :, off * BLK : (off + 1) * BLK] = (sl > -1.0).astype(f4)
        in_maps.append(
            {
                "xT": xTc.astype(bf),
                "xTq": np.ascontiguousarray(qcols).astype(bf),
                "Wall": Wall.astype(bf),
                "P2": P2.astype(bf),
                "cosk": cosk2.astype(bf),
                "sink": sink2.astype(bf),
                "cosq": np.concatenate([cosqc, cosqc], 0).astype(bf),
                "sinq": np.concatenate([sinqc, sinqc], 0).astype(bf),
                "maskst": maskstk.astype(bf),
            }
        )
    return in_maps


def kernel(x, cos, sin, mask, Wq, Wk, Wv, Wo, _trace=False, _trace_kwargs=None):
    from concourse import bass_utils

    x = np.asarray(x)
    in_maps = _make_in_maps(
        np.asarray(x), np.asarray(cos), np.asarray(sin), np.asarray(mask),
        np.asarray(Wq), np.asarray(Wk), np.asarray(Wv), np.asarray(Wo),
    )
    nc = _get_nc()
    kw = {}
    if _trace:
        kw["trace"] = True
        if _trace_kwargs:
            kw.update(_trace_kwargs)
    res = bass_utils.run_bass_kernel_spmd(nc, in_maps, core_ids=list(range(8)), **kw)
    out = np.empty((B, S, H), np.float32)
    for c in range(8):
        b = c // 2
        blocks = BLOCKS_EVEN if c % 2 == 0 else BLOCKS_ODD
        o = np.asarray(res.results[c]["out"]).astype(np.float32)  # [1024, 576]
        for j, blk in enumerate(blocks):
            out[b, blk * BLK : (blk + 1) * BLK, :] = o[j * BLK : (j + 1) * BLK, :]
    if _trace:
        _CACHED["last_result"] = res
    return out
